# revision 1
# baseline (speedup 1.0000x reference)
"""GPTBigCode fused MQA attention block (prefill) on 8 Trainium2 NeuronCores.

Full-problem shapes: x [2,2048,2048], w_attn [2048,2304], w_proj [2048,2048],
H=16 query heads, head_dim=128, 1 shared K/V head (MQA), causal softmax.

Sharding: 2-way data parallel over batch x 4-way tensor parallel over query
heads. Core c handles batch c//4 and query heads 4*(c%4)..4*(c%4)+3; the
single K/V head is replicated. c_proj is row-sharded, so each core emits a
partial [2048,2048] output; the host gather sums the 4 partials per batch
(b_proj is added on exactly one core per batch).

Per-core kernel (all on-chip after the input DMAs):
  qkv^T = w_shard^T @ x^T           (bf16 matmuls, x passed pre-transposed,
                                     fp32 PSUM accumulation; q/k kept f32r)
  scores^T[j] = k_block_j @ q_h^T   (f32r, per 128-row k block, 512-col q chunk)
  probs^T = exp(scores/sqrt(128))   (no max subtraction: |scores| <~ 8 for
                                     unit-normal activations, exp is safe)
  out^T[h] += lhsT=v[j], rhs=probs^T  (bf16, accumulated in PSUM)
  rowsum via ones^T @ acc matmul, normalize out^T by approx 1/rowsum
  partial = out^T_norm^T @ w_proj_shard + b_proj   (bf16)
"""

import os
import sys

for _p in ("/opt/trn_rl_repo", "/root/.axon_site/_ro/trn_rl_repo"):
    if os.path.isdir(_p) and _p not in sys.path:
        sys.path.insert(0, _p)
        break

import numpy as np

B, S, D = 2, 2048, 2048
H, HD = 16, 128
P = 128
NH = 4          # query heads per core
DT = D // P     # 16 contraction tiles
CT = NH + 2     # qkv^T c-tiles per core (4 q heads + k + v)
SC = 512        # qkv phase s-chunk (moving free dim)
NSC = S // SC
QC = 512        # attention q chunk
NQC = S // QC
KB = S // P     # 16 k blocks
SCALE = float(1.0 / np.sqrt(np.float32(HD)))

_cache = {}
_last_results = None


def _build():
    import concourse.mybir as mybir
    import concourse.tile as tile
    from concourse import bacc
    from concourse.masks import make_identity

    F32 = mybir.dt.float32
    F32R = mybir.dt.float32r
    BF16 = mybir.dt.bfloat16
    ADD = mybir.AluOpType.add
    MULT = mybir.AluOpType.mult
    EXP = mybir.ActivationFunctionType.Exp

    nc = bacc.Bacc("TRN2", target_bir_lowering=False, debug=False)

    xT = nc.dram_tensor("xT", [D, S], BF16, kind="ExternalInput").ap()
    wq = nc.dram_tensor("wq", [D, CT * P], BF16, kind="ExternalInput").ap()
    bqkv = nc.dram_tensor("bqkv", [CT, P], F32, kind="ExternalInput").ap()
    wp = nc.dram_tensor("wp", [NH * P, D], BF16, kind="ExternalInput").ap()
    bp = nc.dram_tensor("bp", [1, D], F32, kind="ExternalInput").ap()
    out = nc.dram_tensor("out_p", [S, D], F32, kind="ExternalOutput").ap()

    xT_r = xT.rearrange("(dt p) s -> p dt s", p=P)       # [128, 16, 2048]
    wq_r = wq.rearrange("(dt p) c -> p dt c", p=P)       # [128, 16, 768]
    wp_r = wp.rearrange("(ct p) d -> p ct d", p=P)       # [128, 4, 2048]

    with tile.TileContext(nc) as tc:
        with (
            tc.tile_pool(name="consts", bufs=1) as consts,
            tc.tile_pool(name="xt", bufs=2) as p_xt,
            tc.tile_pool(name="wqp", bufs=1) as p_wq,
            tc.tile_pool(name="qk", bufs=1) as p_qk,
            tc.tile_pool(name="vv", bufs=1) as p_v,
            tc.tile_pool(name="vtmp", bufs=2) as p_vtmp,
            tc.tile_pool(name="probs", bufs=6) as p_probs,
            tc.tile_pool(name="accp", bufs=1) as p_acc,
            tc.tile_pool(name="ot", bufs=2) as p_ot,
            tc.tile_pool(name="wpp", bufs=2) as p_wp,
            tc.tile_pool(name="outsb", bufs=4) as p_out,
            tc.tile_pool(name="bcastp", bufs=2) as p_bc,
            tc.tile_pool(name="pmm", bufs=4, space="PSUM") as pp_mm,
            tc.tile_pool(name="pout", bufs=4, space="PSUM") as pp_out,
        ):
            # ---- constants ----
            ident = consts.tile([P, P], BF16)
            make_identity(nc, ident)
            ones_f32 = consts.tile([P, 1], F32)
            nc.vector.memset(ones_f32, 1.0)
            ones = consts.tile([P, 1], F32R)
            nc.vector.tensor_copy(out=ones, in_=ones_f32)
            bq_sb = consts.tile([P, CT], F32)
            nc.sync.dma_start(out=bq_sb, in_=bqkv.rearrange("c p -> p c"))
            bp_row = consts.tile([1, D], F32)
            nc.sync.dma_start(out=bp_row, in_=bp)
            bp_sb = consts.tile([P, D], F32)
            nc.gpsimd.partition_broadcast(bp_sb, bp_row[0:1, :])

            # ---- weights ----
            wq_t = p_wq.tile([P, DT, CT * P], BF16, name="wq_t")
            nc.sync.dma_start(out=wq_t, in_=wq_r)

            # qkv^T [c, s]: c-tiles 0..3 = q heads, 4 = k^T; v^T is
            # transposed on the fly into v ([s,128] blocks).
            qkT = p_qk.tile([P, NH + 1, S], F32R, name="qkT")
            v = p_v.tile([P, KB, HD], BF16, name="v")

            # ---- phase 1: qkv^T = wq^T @ x^T ----
            for sc in range(NSC):
                ssl = slice(sc * SC, (sc + 1) * SC)
                xt = p_xt.tile([P, DT, SC], BF16, name="xt", tag="xt")
                nc.sync.dma_start(out=xt, in_=xT_r[:, :, ssl])
                for ct in range(CT):
                    ps = pp_mm.tile([P, 512], mybir.dt.float32, tag="mm",
                                    name=f"qkv_ps_{sc}_{ct}")
                    for dt_i in range(DT):
                        nc.tensor.matmul(
                            ps[:, :SC],
                            lhsT=wq_t[:, dt_i, ct * P:(ct + 1) * P],
                            rhs=xt[:, dt_i, :],
                            start=(dt_i == 0),
                            stop=(dt_i == DT - 1),
                        )
                    bias = bq_sb[:, ct:ct + 1].to_broadcast((P, SC))
                    if ct < NH + 1:
                        nc.vector.tensor_tensor(
                            out=qkT[:, ct, ssl], in0=ps[:, :SC], in1=bias, op=ADD)
                    else:
                        # v^T chunk -> transpose 128x128 blocks -> v [s,128]
                        vt = p_vtmp.tile([P, SC], BF16, name="vt", tag="vt")
                        nc.vector.tensor_tensor(
                            out=vt, in0=ps[:, :SC], in1=bias, op=ADD)
                        for i in range(SC // P):
                            pst = pp_mm.tile([P, P], BF16, tag="mm",
                                             name=f"vtr_{sc}_{i}")
                            nc.tensor.transpose(pst, vt[:, i * P:(i + 1) * P], ident)
                            nc.vector.tensor_copy(
                                out=v[:, sc * (SC // P) + i, :], in_=pst)

            # ---- phase 2: attention + c_proj, per 512-wide q chunk ----
            for qi in range(NQC):
                jmax = 4 * qi + 4
                acc = p_acc.tile([P, NH, QC], F32R, name=f"acc_{qi}", tag="acc")
                po = [pp_out.tile([P, QC], mybir.dt.float32, tag="po",
                                  name=f"po_{qi}_{h}") for h in range(NH)]
                for j in range(jmax):
                    t = j - 4 * qi
                    off = max(0, t * P)
                    w = QC - off
                    for h in range(NH):
                        ps = pp_mm.tile([P, 512], mybir.dt.float32, tag="mm",
                                        name=f"sc_ps_{qi}_{j}_{h}")
                        nc.tensor.matmul(
                            ps[:, :w],
                            lhsT=qkT[:, NH, j * P:(j + 1) * P],
                            rhs=qkT[:, h, qi * QC + off:(qi + 1) * QC],
                            start=True, stop=True,
                        )
                        pT = p_probs.tile([P, QC], BF16, name=f"pT_{qi}_{j}_{h}",
                                          tag="pT")
                        nc.scalar.activation(pT[:, :w], ps[:, :w], EXP, scale=SCALE)
                        if t >= 0:
                            # strict causal boundary inside the leading block
                            nc.gpsimd.affine_select(
                                out=pT[:, 0:P], in_=pT[:, 0:P],
                                compare_op=mybir.AluOpType.is_ge,
                                fill=0.0, base=0,
                                pattern=[[1, P]], channel_multiplier=-1,
                            )
                        if j == 0:
                            nc.vector.tensor_copy(out=acc[:, h, :], in_=pT)
                        else:
                            nc.vector.tensor_tensor(
                                out=acc[:, h, off:], in0=acc[:, h, off:],
                                in1=pT[:, :w], op=ADD)
                        nc.tensor.matmul(
                            po[h][:, off:],
                            lhsT=v[:, j, :],
                            rhs=pT[:, :w],
                            start=(j == 0), stop=(j == jmax - 1),
                        )

                oT = p_ot.tile([P, NH, QC], BF16, name=f"oT_{qi}", tag="oT")
                for h in range(NH):
                    pss = pp_mm.tile([1, QC], mybir.dt.float32, tag="mm",
                                     name=f"sum_ps_{qi}_{h}")
                    nc.tensor.matmul(pss, lhsT=ones, rhs=acc[:, h, :],
                                     start=True, stop=True)
                    rec = p_bc.tile([1, QC], F32, name=f"rec_{qi}_{h}", tag="rec")
                    nc.vector.reciprocal_approx_fast(out=rec, in_=pss)
                    bc = p_bc.tile([P, QC], F32, name=f"bc_{qi}_{h}", tag="bc")
                    nc.gpsimd.partition_broadcast(bc, rec[0:1, :])
                    nc.vector.tensor_tensor(
                        out=oT[:, h, :], in0=po[h], in1=bc, op=MULT)

                # c_proj rows for this q chunk
                for dc in range(D // QC):
                    dsl = slice(dc * QC, (dc + 1) * QC)
                    wpt = p_wp.tile([P, NH, QC], BF16, name=f"wpt_{qi}_{dc}",
                                    tag="wpt")
                    nc.sync.dma_start(out=wpt, in_=wp_r[:, :, dsl])
                    for st in range(QC // P):
                        psp = pp_mm.tile([P, 512], mybir.dt.float32, tag="mm",
                                         name=f"pr_ps_{qi}_{dc}_{st}")
                        for h in range(NH):
                            nc.tensor.matmul(
                                psp,
                                lhsT=oT[:, h, st * P:(st + 1) * P],
                                rhs=wpt[:, h, :],
                                start=(h == 0), stop=(h == NH - 1),
                            )
                        ob = p_out.tile([P, QC], F32, name=f"ob_{qi}_{dc}_{st}",
                                        tag="ob")
                        nc.vector.tensor_tensor(
                            out=ob, in0=psp, in1=bp_sb[:, dsl], op=ADD)
                        nc.sync.dma_start(
                            out=out[qi * QC + st * P: qi * QC + (st + 1) * P, dsl],
                            in_=ob)

    nc.compile()
    return nc


def _get_nc():
    if "nc" not in _cache:
        _cache["nc"] = _build()
    return _cache["nc"]


def kernel(x, w_attn, b_attn, w_proj, b_proj, start_pos=0, **_ignored):
    global _last_results
    import ml_dtypes
    from concourse.bass_utils import run_bass_kernel_spmd

    bf16 = ml_dtypes.bfloat16
    x = np.asarray(x, dtype=np.float32)
    w_attn = np.asarray(w_attn, dtype=np.float32)
    b_attn = np.asarray(b_attn, dtype=np.float32)
    w_proj = np.asarray(w_proj, dtype=np.float32)
    b_proj = np.asarray(b_proj, dtype=np.float32)

    nc = _get_nc()

    in_maps = []
    for c in range(8):
        b, hg = divmod(c, 4)
        qcols = slice(hg * NH * HD, (hg + 1) * NH * HD)
        wq_shard = np.ascontiguousarray(
            np.concatenate([w_attn[:, qcols], w_attn[:, D:D + HD],
                            w_attn[:, D + HD:D + 2 * HD]],
                           axis=1).astype(bf16))
        bq_shard = np.ascontiguousarray(
            np.concatenate([b_attn[qcols], b_attn[D:D + HD],
                            b_attn[D + HD:D + 2 * HD]]).reshape(CT, P))
        in_maps.append({
            "xT": np.ascontiguousarray(x[b].T.astype(bf16)),
            "wq": wq_shard,
            "bqkv": bq_shard,
            "wp": np.ascontiguousarray(
                w_proj[hg * NH * HD:(hg + 1) * NH * HD].astype(bf16)),
            "bp": (b_proj if hg == 0 else np.zeros_like(b_proj)).reshape(1, D),
        })

    res = run_bass_kernel_spmd(nc, in_maps, core_ids=list(range(8)))
    _last_results = res
    parts = [r["out_p"] for r in res.results]
    out = np.stack([parts[0] + parts[1] + parts[2] + parts[3],
                    parts[4] + parts[5] + parts[6] + parts[7]]).astype(np.float32)
    return out



# revision 14
# speedup vs baseline: 1.2595x; 1.2595x over previous
"""GPTBigCode fused MQA attention block (prefill) on 8 Trainium2 NeuronCores.

Full-problem shapes: x [2,2048,2048], w_attn [2048,2304], w_proj [2048,2048],
H=16 query heads, head_dim=128, 1 shared K/V head (MQA), causal softmax.

Sharding: 2-way data parallel over batch x 4-way tensor parallel over query
heads. Core c handles batch c//4 and query heads 4*(c%4)..4*(c%4)+3; the
single K/V head is replicated. c_proj is row-sharded, so each core emits a
bf16 partial [2048,2048]; the host sums the 4 partials per batch in f32 and
adds b_proj there.

Per-core kernel (all on-chip after the input DMAs):
  qkv^T = w_shard^T @ x^T            (bf16 matmuls, fp32 PSUM, bias on DVE)
  scores^T[j] = k_block_j @ q_h^T    (bf16, per 128-row k block, 512-col q)
  probs^T = exp(scores/sqrt(128))    (scalar engine -> fp16; no max subtract)
  out^T[h] += lhsT=v[j], rhs=probs^T (fp16, accumulated in PSUM)
  rowsum = ones^T @ acc              (acc = sum_j probs^T; fp16 DVE adds run
                                      in the 4x all-SBUF 16-bit DVE mode)
  1/rowsum broadcast via PE matmul   (ones_col @ rec row)
  partial = (out^T/rowsum)^T @ w_proj_shard  (bf16)

The scalar engine's exp is the pacer of the attention inner loop (~2.2us per
k-block group vs ~1.7us of PE work), so qkv chunks for s-chunk sc>=1 and
c_proj tiles are emitted as *filler* between attention groups from a work
queue; the PE stays saturated while exp runs. qkv fillers for s-chunk sc are
force-drained before q-chunk qi=sc needs them. Host pre-arranges every DRAM
operand so each DMA is per-partition contiguous (4-16KB descriptors).
"""

import os
import sys
from collections import deque

for _p in ("/opt/trn_rl_repo", "/root/.axon_site/_ro/trn_rl_repo"):
    if os.path.isdir(_p) and _p not in sys.path:
        sys.path.insert(0, _p)
        break

import numpy as np

B, S, D = 2, 2048, 2048
H, HD = 16, 128
P = 128
NH = 4           # query heads per core
DT = D // P      # 16 contraction tiles
CT = NH + 2      # qkv c-tiles per core (4 q heads + k + v)
SC = 512         # qkv phase s-chunk
NSC = S // SC
QC = 512         # attention q chunk
NQC = S // QC
KB = S // P      # 16 k blocks
SCALE = float(1.0 / np.sqrt(np.float32(HD)))

_cache = {}
_last_results = None


def _build():
    import concourse.mybir as mybir
    import concourse.tile as tile
    from concourse import bacc
    from concourse.masks import make_identity

    F32 = mybir.dt.float32
    F32R = mybir.dt.float32r
    BF16 = mybir.dt.bfloat16
    FP16 = mybir.dt.float16
    ADD = mybir.AluOpType.add
    MULT = mybir.AluOpType.mult
    EXP = mybir.ActivationFunctionType.Exp

    nc = bacc.Bacc("TRN2", target_bir_lowering=False, debug=False)

    # host-prearranged layouts: every DMA is per-partition contiguous
    xtd = nc.dram_tensor("xt", [NSC, P, DT * SC], BF16, kind="ExternalInput").ap()
    wqd = nc.dram_tensor("wq", [CT, P, DT * P], BF16, kind="ExternalInput").ap()
    bq = nc.dram_tensor("bq", [P, CT], F32, kind="ExternalInput").ap()
    wpd = nc.dram_tensor("wp", [P, NH * D], BF16, kind="ExternalInput").ap()
    outd = nc.dram_tensor("out_p", [S, D], BF16, kind="ExternalOutput").ap()

    with tile.TileContext(nc) as tc:
        with (
            tc.tile_pool(name="consts", bufs=1) as consts,
            tc.tile_pool(name="wqp", bufs=1) as p_wq,
            tc.tile_pool(name="wpp", bufs=1) as p_wp,
            tc.tile_pool(name="xt", bufs=4) as p_xt,
            tc.tile_pool(name="qk", bufs=1) as p_qk,
            tc.tile_pool(name="vv", bufs=1) as p_v,
            tc.tile_pool(name="vtmp", bufs=2) as p_vtmp,
            tc.tile_pool(name="probs", bufs=8) as p_probs,
            tc.tile_pool(name="accp", bufs=2) as p_acc,
            tc.tile_pool(name="ot", bufs=2) as p_ot,
            tc.tile_pool(name="recp", bufs=4) as p_rec,
            tc.tile_pool(name="bcp", bufs=2) as p_bc,
            tc.tile_pool(name="outsb", bufs=4) as p_ob,
            tc.tile_pool(name="pmm", bufs=3, space="PSUM") as pp_mm,
            tc.tile_pool(name="pqkv", bufs=1, space="PSUM") as pp_qkv,
            tc.tile_pool(name="pout", bufs=4, space="PSUM") as pp_out,
        ):
            # ---- constants ----
            ident = consts.tile([P, P], FP16)
            make_identity(nc, ident)
            ones_f32 = consts.tile([P, 1], F32)
            nc.vector.memset(ones_f32, 1.0)
            ones = consts.tile([P, 1], FP16)
            nc.vector.tensor_copy(out=ones, in_=ones_f32)
            bq_sb = consts.tile([P, CT], F32)

            # ---- static weights ----
            wq_t = p_wq.tile([P, CT * DT * P], BF16, name="wq_t")
            wp_t = p_wp.tile([P, NH * D], BF16, name="wp_t")

            # DMA issue order: wq ct0, bq, xt0, wq ct1-5, xt1, wp, xt2-3.
            nc.sync.dma_start(out=wq_t[:, 0:DT * P], in_=wqd[0])
            nc.sync.dma_start(out=bq_sb, in_=bq)
            xts = [p_xt.tile([P, DT * SC], BF16, name=f"xt_{sc}", tag="xt")
                   for sc in range(NSC)]
            nc.sync.dma_start(out=xts[0], in_=xtd[0])
            for ct in range(1, CT):
                nc.sync.dma_start(out=wq_t[:, ct * DT * P:(ct + 1) * DT * P],
                                  in_=wqd[ct])
            nc.sync.dma_start(out=xts[1], in_=xtd[1])
            hw = NH * D // 2
            nc.sync.dma_start(out=wp_t[:, :hw], in_=wpd[:, :hw])
            nc.sync.dma_start(out=wp_t[:, hw:], in_=wpd[:, hw:])
            nc.sync.dma_start(out=xts[2], in_=xtd[2])
            nc.sync.dma_start(out=xts[3], in_=xtd[3])

            # qkv^T: c-tiles 0..3 = q heads, 4 = k^T (all bf16); v is
            # transposed on the fly into fp16 [s,128] blocks.
            qkT = p_qk.tile([P, (NH + 1) * S], BF16, name="qkT")
            v = p_v.tile([P, KB * HD], FP16, name="v")

            # ---- emission helpers ----
            qkv_n = [0]

            def emit_qkv_ct(sc, ct):
                xt = xts[sc]
                # alternate PSUM pools so ct n+1's matmuls don't wait on the
                # DVE bias-add that frees ct n's accumulator
                pool = pp_qkv if qkv_n[0] % 2 == 0 else pp_mm
                qkv_n[0] += 1
                ps = pool.tile([P, 512], F32,
                               tag="qkv" if pool is pp_qkv else "mm",
                               name=f"qkv_ps_{sc}_{ct}")
                for dt_i in range(DT):
                    nc.tensor.matmul(
                        ps,
                        lhsT=wq_t[:, ct * DT * P + dt_i * P:
                                  ct * DT * P + (dt_i + 1) * P],
                        rhs=xt[:, dt_i * SC:(dt_i + 1) * SC],
                        start=(dt_i == 0),
                        stop=(dt_i == DT - 1),
                    )
                bias = bq_sb[:, ct:ct + 1].to_broadcast((P, SC))
                if ct < NH + 1:
                    nc.vector.tensor_tensor(
                        out=qkT[:, ct * S + sc * SC:ct * S + (sc + 1) * SC],
                        in0=ps, in1=bias, op=ADD)
                else:
                    vt = p_vtmp.tile([P, SC], FP16, name=f"vt_{sc}", tag="vt")
                    nc.vector.tensor_tensor(out=vt, in0=ps, in1=bias, op=ADD)
                    for i in range(SC // P):
                        pst = pp_mm.tile([P, P], FP16, tag="mm",
                                         name=f"vtr_{sc}_{i}")
                        nc.tensor.transpose(pst, vt[:, i * P:(i + 1) * P],
                                            ident)
                        jb = sc * (SC // P) + i
                        nc.vector.tensor_copy(
                            out=v[:, jb * P:(jb + 1) * P], in_=pst)

            def emit_tail(qi, acc_t, po_l, oT_t):
                # per-head: rowsum -> 1/x -> pool broadcast -> normalize
                for h in range(NH):
                    pss = pp_mm.tile([1, QC], F32, tag="mm",
                                     name=f"pss_{qi}_{h}")
                    nc.tensor.matmul(pss, lhsT=ones,
                                     rhs=acc_t[:, h * QC:(h + 1) * QC],
                                     start=True, stop=True)
                    rec = p_rec.tile([1, QC], F32, tag="rec",
                                     name=f"rec_{qi}_{h}")
                    nc.vector.reciprocal_approx_fast(out=rec, in_=pss)
                    bc = p_bc.tile([P, QC], F32, tag="bc", name=f"bc_{qi}_{h}")
                    nc.gpsimd.partition_broadcast(bc, rec[0:1, :])
                    nc.vector.tensor_tensor(
                        out=oT_t[:, h * QC:(h + 1) * QC], in0=po_l[h], in1=bc,
                        op=MULT)

            def emit_proj_psp(qi, oT_t, dc, st):
                psp = pp_mm.tile([P, 512], F32, tag="mm",
                                 name=f"pr_{qi}_{dc}_{st}")
                for h in range(NH):
                    nc.tensor.matmul(
                        psp,
                        lhsT=oT_t[:, h * QC + st * P:h * QC + (st + 1) * P],
                        rhs=wp_t[:, h * D + dc * QC:h * D + (dc + 1) * QC],
                        start=(h == 0), stop=(h == NH - 1),
                    )
                ob = p_ob.tile([P, QC], BF16, tag="ob",
                               name=f"ob_{qi}_{dc}_{st}")
                nc.vector.tensor_copy(out=ob, in_=psp)
                nc.sync.dma_start(
                    out=outd[qi * QC + st * P:qi * QC + (st + 1) * P,
                             dc * QC:(dc + 1) * QC],
                    in_=ob)

            # filler queue: ("qkv", sc, ct) | ("proj", qi, oT_t, dc, st)
            fillers = deque()

            def pop_filler():
                if not fillers:
                    return
                if fillers[0][0] == "qkv":
                    _, sc, ct = fillers.popleft()
                    emit_qkv_ct(sc, ct)
                else:
                    for _ in range(2):
                        if not fillers or fillers[0][0] != "proj":
                            break
                        _, qi, oT_t, dc, st = fillers.popleft()
                        emit_proj_psp(qi, oT_t, dc, st)

            def drain_qkv_upto(sc_needed):
                # emit every queued closure up to and incl. the last qkv
                # filler for s-chunks <= sc_needed (FIFO order keeps proj
                # fillers queued before them flowing too)
                last = -1
                for i, f in enumerate(fillers):
                    if f[0] == "qkv" and f[1] <= sc_needed:
                        last = i
                for _ in range(last + 1):
                    f = fillers.popleft()
                    if f[0] == "qkv":
                        emit_qkv_ct(f[1], f[2])
                    else:
                        emit_proj_psp(f[1], f[2], f[3], f[4])

            # ---- phase 1 prologue: s-chunk 0 ----
            for ct in range(CT):
                emit_qkv_ct(0, ct)

            # ---- attention with interleaved fillers ----
            prev = None
            for qi in range(NQC):
                if qi + 1 < NSC:
                    for ct in range(CT):
                        fillers.append(("qkv", qi + 1, ct))
                drain_qkv_upto(qi)
                jmax = 4 * qi + 4
                acc_t = p_acc.tile([P, NH * QC], FP16, name=f"acc_{qi}",
                                   tag="acc")
                oT_t = p_ot.tile([P, NH * QC], BF16, name=f"oT_{qi}", tag="oT")
                po_l = [pp_out.tile([P, QC], F32, tag="po",
                                    name=f"po_{qi}_{h}") for h in range(NH)]
                for j in range(jmax):
                    t = j - 4 * qi
                    off = max(0, t * P)
                    w = QC - off
                    pTs = []
                    for h in range(NH):
                        ps = pp_mm.tile([P, 512], F32, tag="mm",
                                        name=f"sc_ps_{qi}_{j}_{h}")
                        nc.tensor.matmul(
                            ps[:, :w],
                            lhsT=qkT[:, NH * S + j * P:NH * S + (j + 1) * P],
                            rhs=qkT[:, h * S + qi * QC + off:
                                    h * S + (qi + 1) * QC],
                            start=True, stop=True,
                        )
                        pT = p_probs.tile([P, QC], FP16, tag="pT",
                                          name=f"pT_{qi}_{j}_{h}")
                        nc.scalar.activation(pT[:, :w], ps[:, :w], EXP,
                                             scale=SCALE)
                        if t >= 0:
                            # strict causal boundary inside the leading block
                            nc.gpsimd.affine_select(
                                out=pT[:, 0:P], in_=pT[:, 0:P],
                                compare_op=mybir.AluOpType.is_ge,
                                fill=0.0, base=0,
                                pattern=[[1, P]], channel_multiplier=-1,
                            )
                        pTs.append(pT)
                    if j == 0:
                        if prev is not None:
                            emit_tail(*prev)
                            pqi, _, _, poT = prev
                            for dc in range(D // QC):
                                for st in range(QC // P):
                                    fillers.append(("proj", pqi, poT, dc, st))
                    else:
                        pop_filler()
                    for h in range(NH):
                        if j == 0:
                            nc.vector.tensor_copy(
                                out=acc_t[:, h * QC:(h + 1) * QC], in_=pTs[h])
                        else:
                            nc.vector.tensor_tensor(
                                out=acc_t[:, h * QC + off:(h + 1) * QC],
                                in0=acc_t[:, h * QC + off:(h + 1) * QC],
                                in1=pTs[h][:, :w], op=ADD)
                        nc.tensor.matmul(
                            po_l[h][:, off:],
                            lhsT=v[:, j * P:(j + 1) * P],
                            rhs=pTs[h][:, :w],
                            start=(j == 0), stop=(j == jmax - 1),
                        )
                prev = (qi, acc_t, po_l, oT_t)

            # epilogue: last tail first (its DVE chain hides under the
            # leftover proj matmuls), then leftover fillers, then last proj
            emit_tail(*prev)
            while fillers:
                f = fillers.popleft()
                if f[0] == "qkv":
                    emit_qkv_ct(f[1], f[2])
                else:
                    emit_proj_psp(f[1], f[2], f[3], f[4])
            for dc in range(D // QC):
                for st in range(QC // P):
                    emit_proj_psp(NQC - 1, prev[3], dc, st)

    nc.compile()
    return nc


def _get_nc():
    if "nc" not in _cache:
        _cache["nc"] = _build()
    return _cache["nc"]


def _shard_inputs(x, w_attn, b_attn, w_proj):
    import ml_dtypes
    bf16 = ml_dtypes.bfloat16

    in_maps = []
    xts = []
    for b in range(B):
        # [sc, p, dt*512]: per-partition contiguous chunks of x^T
        xt = np.ascontiguousarray(
            x[b].T.reshape(DT, P, NSC, SC).transpose(2, 1, 0, 3)
            .reshape(NSC, P, DT * SC).astype(bf16))
        xts.append(xt)
    for c in range(8):
        b, hg = divmod(c, 4)
        cols = [w_attn[:, (hg * NH + ct) * HD:(hg * NH + ct + 1) * HD]
                for ct in range(NH)]
        cols.append(w_attn[:, D:D + HD])
        cols.append(w_attn[:, D + HD:D + 2 * HD])
        wq = np.stack([c_.reshape(DT, P, P).transpose(1, 0, 2).reshape(P, DT * P)
                       for c_ in cols]).astype(bf16)
        bqv = [b_attn[(hg * NH + ct) * HD:(hg * NH + ct + 1) * HD]
               for ct in range(NH)]
        bqv.append(b_attn[D:D + HD])
        bqv.append(b_attn[D + HD:D + 2 * HD])
        bqv = np.stack(bqv, axis=1)          # [128, 6]
        wp = (w_proj[hg * NH * HD:(hg + 1) * NH * HD]
              .reshape(NH, P, D).transpose(1, 0, 2).reshape(P, NH * D)
              .astype(bf16))
        in_maps.append({
            "xt": xts[b],
            "wq": np.ascontiguousarray(wq),
            "bq": np.ascontiguousarray(bqv.astype(np.float32)),
            "wp": np.ascontiguousarray(wp),
        })
    return in_maps


def kernel(x, w_attn, b_attn, w_proj, b_proj, start_pos=0, **_ignored):
    global _last_results
    from concourse.bass_utils import run_bass_kernel_spmd

    x = np.asarray(x, dtype=np.float32)
    w_attn = np.asarray(w_attn, dtype=np.float32)
    b_attn = np.asarray(b_attn, dtype=np.float32)
    w_proj = np.asarray(w_proj, dtype=np.float32)
    b_proj = np.asarray(b_proj, dtype=np.float32)

    nc = _get_nc()
    in_maps = _shard_inputs(x, w_attn, b_attn, w_proj)
    res = run_bass_kernel_spmd(nc, in_maps, core_ids=list(range(8)))
    _last_results = res
    parts = [r["out_p"].astype(np.float32) for r in res.results]
    out = np.stack([parts[0] + parts[1] + parts[2] + parts[3],
                    parts[4] + parts[5] + parts[6] + parts[7]])
    return (out + b_proj[None, None, :]).astype(np.float32)


# revision 19
# speedup vs baseline: 1.2626x; 1.0025x over previous
"""GPTBigCode fused MQA attention block (prefill) on 8 Trainium2 NeuronCores.

Full-problem shapes: x [2,2048,2048], w_attn [2048,2304], w_proj [2048,2048],
H=16 query heads, head_dim=128, 1 shared K/V head (MQA), causal softmax.

Sharding: 2-way data parallel over batch x 4-way tensor parallel over query
heads. Core c handles batch c//4 and query heads 4*(c%4)..4*(c%4)+3; the
single K/V head is replicated. c_proj is row-sharded, so each core emits a
bf16 partial [2048,2048]; the host sums the 4 partials per batch in f32 and
adds b_proj there.

Per-core kernel (all on-chip after the input DMAs):
  qkv^T = w_shard^T @ x^T            (bf16 matmuls, fp32 PSUM, bias on DVE)
  scores^T[j] = k_block_j @ q_h^T    (bf16, per 128-row k block, 512-col q)
  probs^T = exp(scores/sqrt(128))    (scalar engine -> fp16; no max subtract)
  out^T[h] += lhsT=v[j], rhs=probs^T (fp16, accumulated in PSUM)
  rowsum = ones^T @ acc              (acc = sum_j probs^T; fp16 DVE adds run
                                      in the 4x all-SBUF 16-bit DVE mode)
  1/rowsum broadcast via PE matmul   (ones_col @ rec row)
  partial = (out^T/rowsum)^T @ w_proj_shard  (bf16)

The scalar engine's exp is the pacer of the attention inner loop (~2.2us per
k-block group vs ~1.7us of PE work), so qkv chunks for s-chunk sc>=1 and
c_proj tiles are emitted as *filler* between attention groups from a work
queue; the PE stays saturated while exp runs. qkv fillers for s-chunk sc are
force-drained before q-chunk qi=sc needs them. Host pre-arranges every DRAM
operand so each DMA is per-partition contiguous (4-16KB descriptors).
"""

import os
import sys
from collections import deque

for _p in ("/opt/trn_rl_repo", "/root/.axon_site/_ro/trn_rl_repo"):
    if os.path.isdir(_p) and _p not in sys.path:
        sys.path.insert(0, _p)
        break

import numpy as np

B, S, D = 2, 2048, 2048
H, HD = 16, 128
P = 128
NH = 4           # query heads per core
DT = D // P      # 16 contraction tiles
CT = NH + 2      # qkv c-tiles per core (4 q heads + k + v)
SC = 512         # qkv phase s-chunk
NSC = S // SC
QC = 512         # attention q chunk
NQC = S // QC
KB = S // P      # 16 k blocks
SCALE = float(1.0 / np.sqrt(np.float32(HD)))

_cache = {}
_last_results = None


def _build():
    import concourse.mybir as mybir
    import concourse.tile as tile
    from concourse import bacc
    from concourse.masks import make_identity

    F32 = mybir.dt.float32
    F32R = mybir.dt.float32r
    BF16 = mybir.dt.bfloat16
    FP16 = mybir.dt.float16
    ADD = mybir.AluOpType.add
    MULT = mybir.AluOpType.mult
    EXP = mybir.ActivationFunctionType.Exp

    nc = bacc.Bacc("TRN2", target_bir_lowering=False, debug=False)

    # host-prearranged layouts: every DMA is per-partition contiguous
    xtd = nc.dram_tensor("xt", [NSC, P, DT * SC], BF16, kind="ExternalInput").ap()
    wqd = nc.dram_tensor("wq", [CT, P, DT * P], BF16, kind="ExternalInput").ap()
    bq = nc.dram_tensor("bq", [P, CT], F32, kind="ExternalInput").ap()
    wpd = nc.dram_tensor("wp", [P, NH * D], BF16, kind="ExternalInput").ap()
    outd = nc.dram_tensor("out_p", [S, D], BF16, kind="ExternalOutput").ap()

    with tile.TileContext(nc) as tc:
        with (
            tc.tile_pool(name="consts", bufs=1) as consts,
            tc.tile_pool(name="wqp", bufs=1) as p_wq,
            tc.tile_pool(name="wpp", bufs=1) as p_wp,
            tc.tile_pool(name="xt", bufs=4) as p_xt,
            tc.tile_pool(name="qk", bufs=1) as p_qk,
            tc.tile_pool(name="vv", bufs=1) as p_v,
            tc.tile_pool(name="vtmp", bufs=2) as p_vtmp,
            tc.tile_pool(name="probs", bufs=8) as p_probs,
            tc.tile_pool(name="accp", bufs=2) as p_acc,
            tc.tile_pool(name="ot", bufs=2) as p_ot,
            tc.tile_pool(name="recp", bufs=4) as p_rec,
            tc.tile_pool(name="bcp", bufs=2) as p_bc,
            tc.tile_pool(name="outsb", bufs=4) as p_ob,
            tc.tile_pool(name="pmm", bufs=3, space="PSUM") as pp_mm,
            tc.tile_pool(name="pqkv", bufs=1, space="PSUM") as pp_qkv,
            tc.tile_pool(name="pout", bufs=4, space="PSUM") as pp_out,
        ):
            # ---- constants ----
            ident = consts.tile([P, P], FP16)
            make_identity(nc, ident)
            ones_f32 = consts.tile([P, 1], F32)
            nc.vector.memset(ones_f32, 1.0)
            ones = consts.tile([P, 1], FP16)
            nc.vector.tensor_copy(out=ones, in_=ones_f32)
            bq_sb = consts.tile([P, CT], F32)

            # ---- static weights ----
            wq_t = p_wq.tile([P, CT * DT * P], BF16, name="wq_t")
            wp_t = p_wp.tile([P, NH * D], BF16, name="wp_t")

            # DMA issue order: wq ct0, bq, xt0, wq ct1-5, xt1, wp, xt2-3.
            nc.sync.dma_start(out=wq_t[:, 0:DT * P], in_=wqd[0])
            nc.sync.dma_start(out=bq_sb, in_=bq)
            xts = [p_xt.tile([P, DT * SC], BF16, name=f"xt_{sc}", tag="xt")
                   for sc in range(NSC)]
            xh = DT * SC // 2
            nc.sync.dma_start(out=xts[0][:, :xh], in_=xtd[0][:, :xh])
            nc.sync.dma_start(out=xts[0][:, xh:], in_=xtd[0][:, xh:])
            for ct in range(1, CT):
                nc.sync.dma_start(out=wq_t[:, ct * DT * P:(ct + 1) * DT * P],
                                  in_=wqd[ct])
            nc.sync.dma_start(out=xts[1], in_=xtd[1])
            hw = NH * D // 2
            nc.sync.dma_start(out=wp_t[:, :hw], in_=wpd[:, :hw])
            nc.sync.dma_start(out=wp_t[:, hw:], in_=wpd[:, hw:])
            nc.sync.dma_start(out=xts[2], in_=xtd[2])
            nc.sync.dma_start(out=xts[3], in_=xtd[3])

            # qkv^T: c-tiles 0..3 = q heads, 4 = k^T (all bf16); v is
            # transposed on the fly into fp16 [s,128] blocks.
            qkT = p_qk.tile([P, (NH + 1) * S], BF16, name="qkT")
            v = p_v.tile([P, KB * HD], FP16, name="v")

            # ---- emission helpers ----
            qkv_n = [0]

            def emit_qkv_ct(sc, ct):
                xt = xts[sc]
                # alternate PSUM pools so ct n+1's matmuls don't wait on the
                # DVE bias-add that frees ct n's accumulator
                pool = pp_qkv if qkv_n[0] % 2 == 0 else pp_mm
                qkv_n[0] += 1
                ps = pool.tile([P, 512], F32,
                               tag="qkv" if pool is pp_qkv else "mm",
                               name=f"qkv_ps_{sc}_{ct}")
                for dt_i in range(DT):
                    nc.tensor.matmul(
                        ps,
                        lhsT=wq_t[:, ct * DT * P + dt_i * P:
                                  ct * DT * P + (dt_i + 1) * P],
                        rhs=xt[:, dt_i * SC:(dt_i + 1) * SC],
                        start=(dt_i == 0),
                        stop=(dt_i == DT - 1),
                    )
                bias = bq_sb[:, ct:ct + 1].to_broadcast((P, SC))
                if ct < NH + 1:
                    nc.vector.tensor_tensor(
                        out=qkT[:, ct * S + sc * SC:ct * S + (sc + 1) * SC],
                        in0=ps, in1=bias, op=ADD)
                else:
                    vt = p_vtmp.tile([P, SC], FP16, name=f"vt_{sc}", tag="vt")
                    nc.vector.tensor_tensor(out=vt, in0=ps, in1=bias, op=ADD)
                    for i in range(SC // P):
                        pst = pp_mm.tile([P, P], FP16, tag="mm",
                                         name=f"vtr_{sc}_{i}")
                        nc.tensor.transpose(pst, vt[:, i * P:(i + 1) * P],
                                            ident)
                        jb = sc * (SC // P) + i
                        nc.vector.tensor_copy(
                            out=v[:, jb * P:(jb + 1) * P], in_=pst)

            def emit_tail(qi, acc_t, po_l, oT_t):
                # per-head: rowsum -> 1/x -> pool broadcast -> normalize
                for h in range(NH):
                    pss = pp_mm.tile([1, QC], F32, tag="mm",
                                     name=f"pss_{qi}_{h}")
                    nc.tensor.matmul(pss, lhsT=ones,
                                     rhs=acc_t[:, h * QC:(h + 1) * QC],
                                     start=True, stop=True)
                    rec = p_rec.tile([1, QC], F32, tag="rec",
                                     name=f"rec_{qi}_{h}")
                    nc.vector.reciprocal_approx_fast(out=rec, in_=pss)
                    bc = p_bc.tile([P, QC], F32, tag="bc", name=f"bc_{qi}_{h}")
                    nc.gpsimd.partition_broadcast(bc, rec[0:1, :])
                    nc.vector.tensor_tensor(
                        out=oT_t[:, h * QC:(h + 1) * QC], in0=po_l[h], in1=bc,
                        op=MULT)

            def emit_proj_psp(qi, oT_t, dc, st):
                psp = pp_mm.tile([P, 512], F32, tag="mm",
                                 name=f"pr_{qi}_{dc}_{st}")
                for h in range(NH):
                    nc.tensor.matmul(
                        psp,
                        lhsT=oT_t[:, h * QC + st * P:h * QC + (st + 1) * P],
                        rhs=wp_t[:, h * D + dc * QC:h * D + (dc + 1) * QC],
                        start=(h == 0), stop=(h == NH - 1),
                    )
                ob = p_ob.tile([P, QC], BF16, tag="ob",
                               name=f"ob_{qi}_{dc}_{st}")
                nc.vector.tensor_copy(out=ob, in_=psp)
                nc.sync.dma_start(
                    out=outd[qi * QC + st * P:qi * QC + (st + 1) * P,
                             dc * QC:(dc + 1) * QC],
                    in_=ob)

            # filler queue: ("qkv", sc, ct) | ("proj", qi, oT_t, dc, st)
            fillers = deque()

            def pop_filler():
                # qkv fillers have the earlier deadline (q-chunk qi needs
                # s-chunk qi drained); pull them out of order if queued
                for i, f in enumerate(fillers):
                    if f[0] == "qkv":
                        del fillers[i]
                        emit_qkv_ct(f[1], f[2])
                        return
                for _ in range(3):
                    if not fillers:
                        break
                    _, qi, oT_t, dc, st = fillers.popleft()
                    emit_proj_psp(qi, oT_t, dc, st)

            def drain_qkv_upto(sc_needed):
                # emit every queued closure up to and incl. the last qkv
                # filler for s-chunks <= sc_needed (FIFO order keeps proj
                # fillers queued before them flowing too)
                last = -1
                for i, f in enumerate(fillers):
                    if f[0] == "qkv" and f[1] <= sc_needed:
                        last = i
                for _ in range(last + 1):
                    f = fillers.popleft()
                    if f[0] == "qkv":
                        emit_qkv_ct(f[1], f[2])
                    else:
                        emit_proj_psp(f[1], f[2], f[3], f[4])

            # ---- phase 1 prologue: s-chunk 0 ----
            for ct in range(CT):
                emit_qkv_ct(0, ct)

            # ---- attention with interleaved fillers ----
            for qi in range(NQC):
                if qi + 1 < NSC:
                    for ct in range(CT):
                        fillers.append(("qkv", qi + 1, ct))
                drain_qkv_upto(qi)
                jmax = 4 * qi + 4
                acc_t = p_acc.tile([P, NH * QC], FP16, name=f"acc_{qi}",
                                   tag="acc")
                oT_t = p_ot.tile([P, NH * QC], BF16, name=f"oT_{qi}", tag="oT")
                po_l = [pp_out.tile([P, QC], F32, tag="po",
                                    name=f"po_{qi}_{h}") for h in range(NH)]
                for j in range(jmax):
                    t = j - 4 * qi
                    off = max(0, t * P)
                    w = QC - off
                    pTs = []
                    for h in range(NH):
                        ps = pp_mm.tile([P, 512], F32, tag="mm",
                                        name=f"sc_ps_{qi}_{j}_{h}")
                        nc.tensor.matmul(
                            ps[:, :w],
                            lhsT=qkT[:, NH * S + j * P:NH * S + (j + 1) * P],
                            rhs=qkT[:, h * S + qi * QC + off:
                                    h * S + (qi + 1) * QC],
                            start=True, stop=True,
                        )
                        pT = p_probs.tile([P, QC], FP16, tag="pT",
                                          name=f"pT_{qi}_{j}_{h}")
                        nc.scalar.activation(pT[:, :w], ps[:, :w], EXP,
                                             scale=SCALE)
                        if t >= 0:
                            # strict causal boundary inside the leading block
                            nc.gpsimd.affine_select(
                                out=pT[:, 0:P], in_=pT[:, 0:P],
                                compare_op=mybir.AluOpType.is_ge,
                                fill=0.0, base=0,
                                pattern=[[1, P]], channel_multiplier=-1,
                            )
                        pTs.append(pT)
                    pop_filler()
                    for h in range(NH):
                        if j == 0:
                            nc.vector.tensor_copy(
                                out=acc_t[:, h * QC:(h + 1) * QC], in_=pTs[h])
                        else:
                            nc.vector.tensor_tensor(
                                out=acc_t[:, h * QC + off:(h + 1) * QC],
                                in0=acc_t[:, h * QC + off:(h + 1) * QC],
                                in1=pTs[h][:, :w], op=ADD)
                        nc.tensor.matmul(
                            po_l[h][:, off:],
                            lhsT=v[:, j * P:(j + 1) * P],
                            rhs=pTs[h][:, :w],
                            start=(j == 0), stop=(j == jmax - 1),
                        )
                # the tail's normalize chain (rowsum->recip->broadcast->mult)
                # overlaps the next q-chunk's qkv drain / score groups
                emit_tail(qi, acc_t, po_l, oT_t)
                for dc in range(D // QC):
                    for st in range(QC // P):
                        fillers.append(("proj", qi, oT_t, dc, st))

            # epilogue: leftover proj fillers
            while fillers:
                f = fillers.popleft()
                if f[0] == "qkv":
                    emit_qkv_ct(f[1], f[2])
                else:
                    emit_proj_psp(f[1], f[2], f[3], f[4])

    nc.compile()
    return nc


def _get_nc():
    if "nc" not in _cache:
        _cache["nc"] = _build()
    return _cache["nc"]


def _shard_inputs(x, w_attn, b_attn, w_proj):
    import ml_dtypes
    bf16 = ml_dtypes.bfloat16

    in_maps = []
    xts = []
    for b in range(B):
        # [sc, p, dt*512]: per-partition contiguous chunks of x^T
        xt = np.ascontiguousarray(
            x[b].T.reshape(DT, P, NSC, SC).transpose(2, 1, 0, 3)
            .reshape(NSC, P, DT * SC).astype(bf16))
        xts.append(xt)
    for c in range(8):
        b, hg = divmod(c, 4)
        cols = [w_attn[:, (hg * NH + ct) * HD:(hg * NH + ct + 1) * HD]
                for ct in range(NH)]
        cols.append(w_attn[:, D:D + HD])
        cols.append(w_attn[:, D + HD:D + 2 * HD])
        wq = np.stack([c_.reshape(DT, P, P).transpose(1, 0, 2).reshape(P, DT * P)
                       for c_ in cols]).astype(bf16)
        bqv = [b_attn[(hg * NH + ct) * HD:(hg * NH + ct + 1) * HD]
               for ct in range(NH)]
        bqv.append(b_attn[D:D + HD])
        bqv.append(b_attn[D + HD:D + 2 * HD])
        bqv = np.stack(bqv, axis=1)          # [128, 6]
        wp = (w_proj[hg * NH * HD:(hg + 1) * NH * HD]
              .reshape(NH, P, D).transpose(1, 0, 2).reshape(P, NH * D)
              .astype(bf16))
        in_maps.append({
            "xt": xts[b],
            "wq": np.ascontiguousarray(wq),
            "bq": np.ascontiguousarray(bqv.astype(np.float32)),
            "wp": np.ascontiguousarray(wp),
        })
    return in_maps


def kernel(x, w_attn, b_attn, w_proj, b_proj, start_pos=0, **_ignored):
    global _last_results
    from concourse.bass_utils import run_bass_kernel_spmd

    x = np.asarray(x, dtype=np.float32)
    w_attn = np.asarray(w_attn, dtype=np.float32)
    b_attn = np.asarray(b_attn, dtype=np.float32)
    w_proj = np.asarray(w_proj, dtype=np.float32)
    b_proj = np.asarray(b_proj, dtype=np.float32)

    nc = _get_nc()
    in_maps = _shard_inputs(x, w_attn, b_attn, w_proj)
    res = run_bass_kernel_spmd(nc, in_maps, core_ids=list(range(8)))
    _last_results = res
    parts = [r["out_p"].astype(np.float32) for r in res.results]
    out = np.stack([parts[0] + parts[1] + parts[2] + parts[3],
                    parts[4] + parts[5] + parts[6] + parts[7]])
    return (out + b_proj[None, None, :]).astype(np.float32)
